# revision 4
# baseline (speedup 1.0000x reference)
"""Trainium2 Bass kernel for nn_LoraLayer: out = x @ W^T + b + (alpha/r) * x @ (A @ B).

Sharding (8 cores): 2 token-groups x 4 feature-groups.
  core c: tg = c // 4 (tokens), fg = c % 4 (output features)
Each core computes out[tg_tokens, fg_feats] = x_tg @ Weff_fg + b_fg where
  Weff_fg = W^T[:, fg] + A @ (scale*B)[:, fg]   (folded on device, SBUF-resident)

All matmuls run as float32r (full PE rate, ~1e-4 rel err). Host does layout
prep only (transposes / re-tiling); all FLOPs for the linear+LoRA math happen
on device.
"""
import numpy as np

import concourse.bass as bass
import concourse.mybir as mybir
import concourse.tile as tile
from concourse import bacc
from concourse.bass_utils import run_bass_kernel_spmd

# Problem shape (hardcoded per contract)
B, S = 4, 2048
FIN, FOUT, R = 4096, 4096, 16
ALPHA = 32.0
SCALE = ALPHA / R  # 2.0
TOKENS = B * S  # 8192

N_CORES = 8
TG, FG = 2, 4  # token groups x feature groups
TOK_SH = TOKENS // TG  # 4096 tokens per core
F_SH = FOUT // FG  # 1024 out features per core
P = 128
KT = FIN // P  # 32 contraction tiles
MT = TOK_SH // P  # 32 token tiles per core
NSL = F_SH // 512  # 2 psum-wide slices

_CACHED_NC = None


def _build_nc(mt=MT, kt=KT, debug=False):
    f32 = mybir.dt.float32
    f32r = mybir.dt.float32r
    nc = bacc.Bacc(None, target_bir_lowering=False, debug=debug)

    # xt: per m-tile contiguous layout [MT, P, KT*P]; xt[m, p, ko*P + mm]
    #   = x[token m*P+mm, feature ko*P+p]
    xt_d = nc.declare_dram_parameter("xt", [mt, P, kt * P], f32r, isOutput=False)
    # wt: W^T shard [kt * P, F_SH]
    wt_d = nc.declare_dram_parameter("wt", [kt * P, F_SH], f32r, isOutput=False)
    # at: lora_a^T [R, kt * P]
    at_d = nc.declare_dram_parameter("at", [R, kt * P], f32r, isOutput=False)
    # bs: scale * lora_b shard [R, F_SH]
    bs_d = nc.declare_dram_parameter("bs", [R, F_SH], f32r, isOutput=False)
    # bias shard [F_SH]
    b_d = nc.declare_dram_parameter("bias", [F_SH], f32, isOutput=False)
    out_d = nc.declare_dram_parameter("out", [mt * P, F_SH], f32, isOutput=True)

    wt_t = wt_d[:].rearrange("(ko p) n -> p ko n", p=P)  # [P, KT, F_SH]
    out_t = out_d[:].rearrange("(mo p) n -> p mo n", p=P)  # [P, MT, F_SH]

    with tile.TileContext(nc) as tc:
        with (
            tc.tile_pool(name="const", bufs=1) as const,
            tc.tile_pool(name="apool", bufs=3) as apool,
            tc.tile_pool(name="xpool", bufs=3) as xpool,
            tc.tile_pool(name="opool", bufs=3) as opool,
            tc.tile_pool(name="psum", bufs=4, space="PSUM") as psum,
        ):
            # bias broadcast across partitions: [P, F_SH]
            bias_sb = const.tile([P, F_SH], f32)
            nc.sync.dma_start(bias_sb[:], b_d[:].partition_broadcast(P))

            # lora B (pre-scaled) resident: [R, F_SH]
            bs_sb = const.tile([R, F_SH], f32r)
            nc.sync.dma_start(bs_sb[:], bs_d[:])

            # W_eff resident: [P, KT, F_SH] = 128KB/partition
            weff = const.tile([P, kt, F_SH], f32r)

            # Fold: weff[:, ki, :] = wt[:, kt, :] + at[:, kt-slice].T @ bs
            for ki in range(kt):
                nc.sync.dma_start(weff[:, ki, :], wt_t[:, ki, :])
                a_t = apool.tile([R, P], f32r)
                nc.sync.dma_start(a_t[:], at_d[:, ki * P:(ki + 1) * P])
                for ns in range(NSL):
                    pf = psum.tile([P, 512], f32, tag="acc")
                    nc.tensor.matmul(
                        pf[:],
                        a_t[:],
                        bs_sb[:, ns * 512:(ns + 1) * 512],
                        start=True,
                        stop=True,
                    )
                    nc.vector.tensor_add(
                        out=weff[:, ki, ns * 512:(ns + 1) * 512],
                        in0=weff[:, ki, ns * 512:(ns + 1) * 512],
                        in1=pf[:],
                    )

            # Main sweep: for each 128-token tile, accumulate over KT k-tiles
            for m in range(mt):
                xm = xpool.tile([P, kt, P], f32r)
                nc.sync.dma_start(xm[:], xt_d[m].rearrange("p (ko mm) -> p ko mm", ko=kt))
                ot = opool.tile([P, F_SH], f32)
                for ns in range(NSL):
                    pt = psum.tile([P, 512], f32, tag="acc")
                    for k in range(kt):
                        nc.tensor.matmul(
                            pt[:],
                            xm[:, k, :],
                            weff[:, k, ns * 512:(ns + 1) * 512],
                            start=(k == 0),
                            stop=(k == kt - 1),
                        )
                    nc.vector.tensor_add(
                        out=ot[:, ns * 512:(ns + 1) * 512],
                        in0=pt[:],
                        in1=bias_sb[:, ns * 512:(ns + 1) * 512],
                    )
                nc.sync.dma_start(out_t[:, m, :], ot[:])

    nc.compile()
    return nc


def _get_nc():
    global _CACHED_NC
    if _CACHED_NC is None:
        _CACHED_NC = _build_nc()
    return _CACHED_NC


def kernel(x, w, b, lora_a, lora_b, _return_exec_info=False):
    x = np.asarray(x, dtype=np.float32)
    w = np.asarray(w, dtype=np.float32)
    b = np.asarray(b, dtype=np.float32)
    lora_a = np.asarray(lora_a, dtype=np.float32)
    lora_b = np.asarray(lora_b, dtype=np.float32)

    x_flat = x.reshape(TOKENS, FIN)
    # xt layout per token-group: [MT, P, KT*P], xt[m, p, ko*P+mm] = x[m*P+mm, ko*P+p]
    x5 = x_flat.reshape(TG, MT, P, KT, P)  # [tg, m, mm, ko, p]
    xh = np.ascontiguousarray(x5.transpose(0, 1, 4, 3, 2)).reshape(TG, MT, P, KT * P)

    wT = np.ascontiguousarray(w.T)  # [FIN, FOUT]
    aT = np.ascontiguousarray(lora_a.T)  # [R, FIN]
    bscaled = np.ascontiguousarray(SCALE * lora_b)  # [R, FOUT]

    in_maps = []
    for c in range(N_CORES):
        tg, fg = c // FG, c % FG
        fsl = slice(fg * F_SH, (fg + 1) * F_SH)
        in_maps.append({
            "xt": xh[tg],
            "wt": np.ascontiguousarray(wT[:, fsl]),
            "at": aT,
            "bs": np.ascontiguousarray(bscaled[:, fsl]),
            "bias": np.ascontiguousarray(b[fsl]),
        })

    nc = _get_nc()
    res = run_bass_kernel_spmd(
        nc, in_maps, core_ids=list(range(N_CORES)), trace=_return_exec_info
    )

    out = np.empty((TOKENS, FOUT), dtype=np.float32)
    for c in range(N_CORES):
        tg, fg = c // FG, c % FG
        out[tg * TOK_SH:(tg + 1) * TOK_SH, fg * F_SH:(fg + 1) * F_SH] = (
            res.results[c]["out"]
        )
    out = out.reshape(B, S, FOUT)
    if _return_exec_info:
        return out, res
    return out


# revision 10
# speedup vs baseline: 1.0090x; 1.0090x over previous
"""Trainium2 Bass kernel for nn_LoraLayer: out = x @ W^T + b + (alpha/r) * x @ (A @ B).

Sharding (8 cores): 2 token-groups x 4 feature-groups.
  core c: tg = c // 4 (tokens), fg = c % 4 (output features)
Each core computes out[tg_tokens, fg_feats] = x_tg @ Weff_fg + b_fg where
  Weff_fg = W^T[:, fg] + A @ (scale*B)[:, fg]   (folded on device, SBUF-resident)

All matmuls run as float32r (full PE rate, ~1e-4 rel err). Host does layout
prep only (transposes / re-tiling); all FLOPs for the linear+LoRA math happen
on device.
"""
import numpy as np

import concourse.bass as bass
import concourse.mybir as mybir
import concourse.tile as tile
from concourse import bacc
from concourse.bass_utils import run_bass_kernel_spmd

# Problem shape (hardcoded per contract)
B, S = 4, 2048
FIN, FOUT, R = 4096, 4096, 16
ALPHA = 32.0
SCALE = ALPHA / R  # 2.0
TOKENS = B * S  # 8192

N_CORES = 8
TG, FG = 2, 4  # token groups x feature groups
TOK_SH = TOKENS // TG  # 4096 tokens per core
F_SH = FOUT // FG  # 1024 out features per core
P = 128
KT = FIN // P  # 32 contraction tiles
MT = TOK_SH // P  # 32 token tiles per core
NSL = F_SH // 512  # 2 psum-wide slices

_CACHED_NC = None


def _build_nc(mt=MT, kt=KT, debug=False):
    f32 = mybir.dt.float32
    f32r = mybir.dt.float32r
    nc = bacc.Bacc(None, target_bir_lowering=False, debug=debug)

    # xt: per m-tile contiguous layout [MT, P, KT*P]; xt[m, p, ko*P + mm]
    #   = x[token m*P+mm, feature ko*P+p]
    xt_d = nc.declare_dram_parameter("xt", [mt, P, kt * P], f32r, isOutput=False)
    # wt: W^T shard [kt * P, F_SH]
    wt_d = nc.declare_dram_parameter("wt", [kt * P, F_SH], f32r, isOutput=False)
    # at: lora_a^T [R, kt * P]
    at_d = nc.declare_dram_parameter("at", [R, kt * P], f32r, isOutput=False)
    # bs: scale * lora_b shard [R, F_SH]
    bs_d = nc.declare_dram_parameter("bs", [R, F_SH], f32r, isOutput=False)
    # bias shard [F_SH]
    b_d = nc.declare_dram_parameter("bias", [P, F_SH], f32, isOutput=False)
    out_d = nc.declare_dram_parameter("out", [mt * P, F_SH], f32, isOutput=True)

    wt_t = wt_d[:].rearrange("(ko p) n -> p ko n", p=P)  # [P, KT, F_SH]
    out_t = out_d[:].rearrange("(mo p) n -> p mo n", p=P)  # [P, MT, F_SH]

    with tile.TileContext(nc) as tc:
        with (
            tc.tile_pool(name="const", bufs=1) as const,
            tc.tile_pool(name="xpool", bufs=2) as xpool,
            tc.tile_pool(name="opool", bufs=3) as opool,
            tc.tile_pool(name="psum", bufs=4, space="PSUM") as psum,
        ):
            # lora B (pre-scaled) resident: [R, F_SH] (first: fold MMs need it)
            bs_sb = const.tile([R, F_SH], f32r)
            nc.sync.dma_start(bs_sb[:], bs_d[:])

            # bias pre-broadcast on host to [P, F_SH]: plain contiguous DMA
            bias_sb = const.tile([P, F_SH], f32)
            nc.sync.dma_start(bias_sb[:], b_d[:])

            # W_eff resident: [P, KT, F_SH] = 128KB/partition
            weff = const.tile([P, kt, F_SH], f32r)

            # all of lora A first (256KB total) so fold MMs start immediately
            a_sb = const.tile([R, kt, P], f32r)
            nc.sync.dma_start(a_sb[:], at_d[:].rearrange("r (ko p) -> r ko p", ko=kt))

            # wt -> weff in 2MB chunks (dispatch-cheap, fold chases the wavefront)
            WCH = 4 if kt % 4 == 0 else 1
            for kc in range(0, kt, WCH):
                nc.sync.dma_start(
                    weff[:, kc:kc + WCH, :], wt_t[:, kc:kc + WCH, :]
                )

            # Fold: weff[:, ki, :] += at[:, ki-slice].T @ bs
            for ki in range(kt):
                a_t = a_sb[:, ki, :]
                for ns in range(NSL):
                    pf = psum.tile([P, 512], f32, tag="acc")
                    nc.tensor.matmul(
                        pf[:],
                        a_t[:],
                        bs_sb[:, ns * 512:(ns + 1) * 512],
                        start=True,
                        stop=True,
                    )
                    nc.vector.tensor_add(
                        out=weff[:, ki, ns * 512:(ns + 1) * 512],
                        in0=weff[:, ki, ns * 512:(ns + 1) * 512],
                        in1=pf[:],
                    )

            # Main sweep: for each 128-token tile, accumulate over KT k-tiles
            for m in range(mt):
                xm = xpool.tile([P, kt, P], f32r)
                nc.scalar.dma_start(xm[:], xt_d[m].rearrange("p (ko mm) -> p ko mm", ko=kt))
                ot = opool.tile([P, F_SH], f32)
                for ns in range(NSL):
                    pt = psum.tile([P, 512], f32, tag="acc")
                    for k in range(kt):
                        nc.tensor.matmul(
                            pt[:],
                            xm[:, k, :],
                            weff[:, k, ns * 512:(ns + 1) * 512],
                            start=(k == 0),
                            stop=(k == kt - 1),
                        )
                    nc.vector.tensor_add(
                        out=ot[:, ns * 512:(ns + 1) * 512],
                        in0=pt[:],
                        in1=bias_sb[:, ns * 512:(ns + 1) * 512],
                    )
                nc.sync.dma_start(out_t[:, m, :], ot[:])

    nc.compile()
    return nc


def _get_nc():
    global _CACHED_NC
    if _CACHED_NC is None:
        _CACHED_NC = _build_nc()
    return _CACHED_NC


def kernel(x, w, b, lora_a, lora_b, _return_exec_info=False):
    x = np.asarray(x, dtype=np.float32)
    w = np.asarray(w, dtype=np.float32)
    b = np.asarray(b, dtype=np.float32)
    lora_a = np.asarray(lora_a, dtype=np.float32)
    lora_b = np.asarray(lora_b, dtype=np.float32)

    x_flat = x.reshape(TOKENS, FIN)
    # xt layout per token-group: [MT, P, KT*P], xt[m, p, ko*P+mm] = x[m*P+mm, ko*P+p]
    x5 = x_flat.reshape(TG, MT, P, KT, P)  # [tg, m, mm, ko, p]
    xh = np.ascontiguousarray(x5.transpose(0, 1, 4, 3, 2)).reshape(TG, MT, P, KT * P)

    wT = np.ascontiguousarray(w.T)  # [FIN, FOUT]
    aT = np.ascontiguousarray(lora_a.T)  # [R, FIN]
    bscaled = np.ascontiguousarray(SCALE * lora_b)  # [R, FOUT]

    in_maps = []
    for c in range(N_CORES):
        tg, fg = c // FG, c % FG
        fsl = slice(fg * F_SH, (fg + 1) * F_SH)
        in_maps.append({
            "xt": xh[tg],
            "wt": np.ascontiguousarray(wT[:, fsl]),
            "at": aT,
            "bs": np.ascontiguousarray(bscaled[:, fsl]),
            "bias": np.ascontiguousarray(np.broadcast_to(b[fsl][None, :], (P, F_SH))),
        })

    nc = _get_nc()
    res = run_bass_kernel_spmd(
        nc, in_maps, core_ids=list(range(N_CORES)), trace=_return_exec_info
    )

    out = np.empty((TOKENS, FOUT), dtype=np.float32)
    for c in range(N_CORES):
        tg, fg = c // FG, c % FG
        out[tg * TOK_SH:(tg + 1) * TOK_SH, fg * F_SH:(fg + 1) * F_SH] = (
            res.results[c]["out"]
        )
    out = out.reshape(B, S, FOUT)
    if _return_exec_info:
        return out, res
    return out
